# revision 15
# baseline (speedup 1.0000x reference)
# Trainium2 Bass kernel for nn_Attention_35433480192757
#
# reference computation (b=4, c=128, h=w=64, n=h*w=4096):
#   GroupNorm(8, c) -> 1x1 conv qkv -> full [n, n] attention per batch
#   -> 1x1 conv proj -> residual add
#
# Sharding: 8 cores = 4 batches x 2 query-row halves. Each core computes the
# full k/v for its batch (cheap) and attention for its 2048 query rows.
#
# Speed strategy (vs the fp32r baseline):
#   - All n^2 matmuls (scores, PV, softmax-sum) run in fp8 with the PE's
#     DoubleRow perf mode: two 128-deep contraction tiles per pass at
#     0.5 cycles/row -> 4x the fp32r rate.
#   - q/k are stored e4m3 in a [64, 2, n] split-contraction layout so the
#     c=128 contraction presents as two 64-deep tiles.
#   - P = exp(scores) is stored e5m2 (huge exponent range; 2-bit mantissa
#     rounding ~ +-12% per element cancels in the softmax ratio).
#   - exp work is split across three engines: ACT (native Exp -> e5m2),
#     and DVE/Pool via a one-op Schraudolph bit trick:
#       int8( 4*log2(e)*scale * raw + 59.783 ) reinterpreted as e5m2.
#   - GroupNorm is folded into the qkv weights (as in baseline); v bias is
#     folded into the proj bias on device: pb_eff = pb + wp @ bv.
#   - elementwise work is spread over DVE and Pool (gpsimd).

import numpy as np
from contextlib import ExitStack

import concourse.bass as bass
from concourse import bacc
import concourse.tile as tile
import concourse.mybir as mybir
from concourse.bass import ts
from concourse.bass_utils import run_bass_kernel_spmd

P = 128          # partitions == channels
C = 128
HALF = 64        # contraction half for DoubleRow
N = 4096         # sequence length (h*w) per batch
NH = 2048        # query rows per core
CH = 512         # free-dim chunk (one PSUM bank of fp32)
NCH = N // CH    # 8 column chunks of x
NQCH = NH // CH  # 4 column chunks of xq
NJC = N // P     # 32 key chunks (contraction over j)
NIB = NH // CH   # 4 i-blocks per core
NPAIR = NJC // 2  # 16 key-chunk pairs per i-block
NUM_GROUPS = 8
GSIZE = C // NUM_GROUPS
EPS = 1e-5
SCALE = float(C) ** -0.5

F32 = mybir.dt.float32
F32R = mybir.dt.float32r
E4 = mybir.dt.float8e4
E5 = mybir.dt.float8e5
I8 = mybir.dt.int8
AOP = mybir.AluOpType
AFT = mybir.ActivationFunctionType
DR = mybir.MatmulPerfMode.DoubleRow

# e5m2 exp bit trick: bits = round(A_E * raw_score + B_E), bits as e5m2.
# A_E folds in the 1/sqrt(c) softmax scale; B_E centres the multiplicative
# sawtooth so it matches the ACT exact-exp-then-round-to-e5m2 path.
A_E = float(4.0 * np.log2(np.e) * SCALE)
B_E = 60.0 - 0.2171

# exp engine schedule per i-block: 16 pair-slots, each 'A' (ACT) or 'D'
# (DVE). GPSIMD cannot touch PSUM on TRN2, so only these two engines can
# read the score banks. ACT owns the k/v converts (preamble-heavy), so the
# first i-block leans on DVE.
SCHED = [
    "DDDDDADDADDADADA",   # ib0: A5 D11
    "AADADADADADAADAA",   # ib1: A9 D7
    "AADAADADADAADAAD",   # ib2: A10 D6
    "ADAADAADAADAAADA",   # ib3: A10 D6
]


def _build_program(reps=1):
    nc = bacc.Bacc(trn_type="TRN2", num_devices=8)

    x_d = nc.dram_tensor("x", [P, N], F32R, kind="ExternalInput")
    xq_d = nc.dram_tensor("xq", [P, NH], F32R, kind="ExternalInput")
    wqT_d = nc.dram_tensor("wqT", [P, P], F32, kind="ExternalInput")
    wkT_d = nc.dram_tensor("wkT", [P, P], F32, kind="ExternalInput")
    wvT_d = nc.dram_tensor("wvT", [P, P], F32, kind="ExternalInput")
    wpT_d = nc.dram_tensor("wpT", [P, P], F32R, kind="ExternalInput")
    qkvb_d = nc.dram_tensor("qkvb", [P, 2], F32, kind="ExternalInput")
    bvq_d = nc.dram_tensor("bvq", [P, 1], F32, kind="ExternalInput")
    pb_d = nc.dram_tensor("pb", [P, 1], F32, kind="ExternalInput")
    gnw_d = nc.dram_tensor("gnw", [P, 1], F32, kind="ExternalInput")
    gnb_d = nc.dram_tensor("gnb", [P, 1], F32, kind="ExternalInput")
    out_d = nc.dram_tensor("out", [P, NH], F32, kind="ExternalOutput")

    # constants baked into the NEFF
    gmat_np = np.zeros((P, P), dtype=np.float32)
    for g in range(NUM_GROUPS):
        gmat_np[g * GSIZE:(g + 1) * GSIZE, g * GSIZE:(g + 1) * GSIZE] = 1.0 / GSIZE
    gmat_d = nc.inline_tensor(gmat_np, "gmat")

    with ExitStack() as ctx:
        tc = ctx.enter_context(tile.TileContext(nc))

        consts = ctx.enter_context(tc.tile_pool(name="consts", bufs=1))
        bigs = ctx.enter_context(tc.tile_pool(name="bigs", bufs=1))
        work = ctx.enter_context(tc.tile_pool(name="work", bufs=2))
        small = ctx.enter_context(tc.tile_pool(name="small", bufs=1))
        outp = ctx.enter_context(tc.tile_pool(name="outp", bufs=2))
        psb = ctx.enter_context(tc.tile_pool(name="psb", bufs=3, space="PSUM"))
        psacc = ctx.enter_context(tc.tile_pool(name="psacc", bufs=1, space="PSUM"))
        pssum = ctx.enter_context(tc.tile_pool(name="pssum", bufs=1, space="PSUM"))
        pools = (consts, bigs, work, small, outp, psb, psacc, pssum)
        drams = (x_d, xq_d, wqT_d, wkT_d, wvT_d, wpT_d, qkvb_d, bvq_d,
                 pb_d, gnw_d, gnb_d, gmat_d, out_d)

        for _rep in range(reps):
            _emit_body(nc, pools, drams)

    nc.compile()
    return nc


def _emit_body(nc, pools, drams):
    (consts, bigs, work, small, outp, psb, psacc, pssum) = pools
    (x_d, xq_d, wqT_d, wkT_d, wvT_d, wpT_d, qkvb_d, bvq_d,
     pb_d, gnw_d, gnb_d, gmat_d, out_d) = drams

    # ---------------- loads ----------------
    x_sb = bigs.tile([P, N], F32R, tag="x", name="x_sb")
    for s in range(NCH):
        nc.sync.dma_start(x_sb[:, ts(s, CH)], x_d.ap()[:, ts(s, CH)])
    xq_sb = bigs.tile([P, NH], F32R, tag="xq", name="xq_sb")
    for s in range(NQCH):
        nc.sync.dma_start(xq_sb[:, ts(s, CH)], xq_d.ap()[:, ts(s, CH)])

    wq = consts.tile([P, P], F32, tag="wq", name="wq")
    nc.sync.dma_start(wq[:], wqT_d.ap())
    wk = consts.tile([P, P], F32, tag="wk", name="wk")
    nc.sync.dma_start(wk[:], wkT_d.ap())
    wv = consts.tile([P, P], F32, tag="wv", name="wv")
    nc.sync.dma_start(wv[:], wvT_d.ap())
    wp = consts.tile([P, P], F32R, tag="wp", name="wp")
    nc.sync.dma_start(wp[:], wpT_d.ap())
    qkvb = consts.tile([P, 2], F32, tag="qkvb", name="qkvb")
    nc.sync.dma_start(qkvb[:], qkvb_d.ap())
    bvq = consts.tile([P, 1], F32, tag="bvq", name="bvq")
    nc.sync.dma_start(bvq[:], bvq_d.ap())
    pb = consts.tile([P, 1], F32, tag="pb", name="pb")
    nc.sync.dma_start(pb[:], pb_d.ap())
    gnw = consts.tile([P, 1], F32, tag="gnw", name="gnw")
    nc.sync.dma_start(gnw[:], gnw_d.ap())
    gnb = consts.tile([P, 1], F32, tag="gnb", name="gnb")
    nc.sync.dma_start(gnb[:], gnb_d.ap())
    # prefetch the Ln and Exp activation tables during the x DMA so the
    # loads (1.3us each) stay off the critical path
    tdum = small.tile([P, 1], F32, tag="tdum", name="tdum")
    nc.scalar.activation(tdum[:], gnw[:], AFT.Ln)
    nc.scalar.activation(tdum[:], gnw[:], AFT.Exp)
    ones8 = consts.tile([P, 2, P], E5, tag="ones8", name="ones8")
    nc.vector.memset(ones8[:], 1.0)
    # gmat is DMA'd last; the warmup matmul below then observes the DMA-queue
    # semaphore once, so later matmuls need at most one wait (walrus codegen
    # allows only one sync-wait on a self-loading fp32 matmul).
    gmat = consts.tile([P, P], F32, tag="gmat", name="gmat")
    nc.sync.dma_start(gmat[:], gmat_d.ap())

    # PE warmup: absorb the DMA semaphore wait (see note above).
    ps_t = pssum.tile([P, 12], F32, tag="sp", name="ps_t")
    nc.tensor.matmul(ps_t[:, 8:10], lhsT=gmat[:], rhs=gmat[:, 0:2])

    # ---------------- GroupNorm stats ----------------
    stats = small.tile([P, NCH, 6], F32, tag="stats", name="stats")
    for s in range(NCH):
        nc.vector.bn_stats(stats[:, s, :], x_sb[:, ts(s, CH)])
    mv = small.tile([P, 2], F32, tag="mv", name="mv")  # per-channel mean, var
    nc.vector.bn_aggr(mv[:], stats[:])

    # t2 = [mean_c, E[x^2]_c]
    t2 = small.tile([P, 2], F32, tag="t2", name="t2")
    nc.vector.tensor_mul(t2[:, 1:2], mv[:, 0:1], mv[:, 0:1])
    nc.vector.tensor_add(t2[:, 1:2], t2[:, 1:2], mv[:, 1:2])
    nc.vector.tensor_copy(t2[:, 0:1], mv[:, 0:1])

    # group-average both stats with the block-diagonal averaging matrix
    nc.tensor.matmul(ps_t[:, 0:2], lhsT=gmat[:], rhs=t2[:])
    gstat = small.tile([P, 2], F32, tag="gstat", name="gstat")
    nc.vector.tensor_copy(gstat[:], ps_t[:, 0:2])

    varv = small.tile([P, 1], F32, tag="varv", name="varv")  # var_g + eps
    nc.vector.tensor_mul(varv[:], gstat[:, 0:1], gstat[:, 0:1])
    nc.vector.tensor_sub(varv[:], gstat[:, 1:2], varv[:])
    nc.vector.tensor_scalar_add(varv[:], varv[:], EPS)

    rstd = small.tile([P, 1], F32, tag="rstd", name="rstd")
    lnv = small.tile([P, 1], F32, tag="lnv", name="lnv")
    nc.scalar.activation(lnv[:], varv[:], AFT.Ln)
    nc.scalar.activation(rstd[:], lnv[:], AFT.Exp, scale=-0.5)
    # two Newton steps to clean up the ACT sqrt approximation:
    # y <- y * (1.5 - 0.5 * v * y * y)
    for it in range(2):
        nt = small.tile([P, 1], F32, tag="nt", name=f"nt{it}")
        nc.vector.tensor_mul(nt[:], rstd[:], rstd[:])
        nc.vector.tensor_mul(nt[:], nt[:], varv[:])
        nc.vector.tensor_scalar(nt[:], nt[:], -0.5, 1.5, AOP.mult, AOP.add)
        nc.vector.tensor_mul(rstd[:], rstd[:], nt[:])

    s_c = small.tile([P, 1], F32, tag="s_c", name="s_c")  # per-channel scale
    nc.vector.tensor_mul(s_c[:], rstd[:], gnw[:])
    t_c = small.tile([P, 1], F32, tag="t_c", name="t_c")  # per-channel shift
    nc.vector.tensor_mul(t_c[:], gstat[:, 0:1], s_c[:])
    nc.vector.tensor_sub(t_c[:], gnb[:], t_c[:])

    # ---------------- fold GN into qkv weights ----------------
    wq_s = consts.tile([P, P], F32R, tag="wq_s", name="wq_s")
    nc.vector.tensor_scalar_mul(wq_s[:], wq[:], s_c[:])
    wk_s = consts.tile([P, P], F32R, tag="wk_s", name="wk_s")
    nc.vector.tensor_scalar_mul(wk_s[:], wk[:], s_c[:])
    wv_s = consts.tile([P, P], F32R, tag="wv_s", name="wv_s")
    nc.vector.tensor_scalar_mul(wv_s[:], wv[:], s_c[:])

    # biases: b_{q,k,v}[o] = (W^T t_c)[o] + qkv_bias[o]
    nc.tensor.matmul(ps_t[:, 2:3], lhsT=wq[:], rhs=t_c[:])
    nc.tensor.matmul(ps_t[:, 3:4], lhsT=wk[:], rhs=t_c[:])
    nc.tensor.matmul(ps_t[:, 6:7], lhsT=wv[:], rhs=t_c[:])
    bq = small.tile([P, 1], F32, tag="bq", name="bq")
    nc.vector.tensor_add(bq[:], ps_t[:, 2:3], qkvb[:, 0:1])
    bk = small.tile([P, 1], F32, tag="bk", name="bk")
    nc.vector.tensor_add(bk[:], ps_t[:, 3:4], qkvb[:, 1:2])
    bv = small.tile([P, 1], F32, tag="bv", name="bv")
    nc.vector.tensor_add(bv[:], ps_t[:, 6:7], bvq[:])
    # fold v bias into proj bias: pb_eff = pb + wp @ bv (plain-f32 matmul)
    wpF = consts.tile([P, P], F32, tag="wpF", name="wpF")
    nc.vector.tensor_copy(wpF[:], wp[:])
    nc.tensor.matmul(ps_t[:, 7:8], lhsT=wpF[:], rhs=bv[:])
    pbe = small.tile([P, 1], F32, tag="pbe", name="pbe")
    nc.vector.tensor_add(pbe[:], ps_t[:, 7:8], pb[:])

    # ---------------- qkv projections ----------------
    # kT8/qT8 e4m3 in [c, n] layout, then SBUF->SBUF DMA repartition into
    # the [64, 2, n] split-contraction layout DoubleRow needs.
    kT8 = bigs.tile([P, N], E4, tag="kT8", name="kT8")
    k8 = bigs.tile([HALF, 2, N], E4, tag="k8", name="k8")
    for s in range(NCH // 2):
        psk = psb.tile([P, 2, CH], F32, tag="sc", name=f"psk{s}")
        for hh in range(2):
            ch = 2 * s + hh
            nc.tensor.matmul(psk[:, hh, :], lhsT=wk_s[:],
                             rhs=x_sb[:, ts(ch, CH)])
            # bias-add + e4m3 convert on ACT (Identity allows an AP bias)
            nc.scalar.activation(kT8[:, ts(ch, CH)], psk[:, hh, :],
                                 AFT.Identity, bias=bk[:, 0:1])
            nc.sync.dma_start(k8[:, 0, ts(ch, CH)], kT8[0:HALF, ts(ch, CH)])
            nc.sync.dma_start(k8[:, 1, ts(ch, CH)], kT8[HALF:P, ts(ch, CH)])
    qT8 = bigs.tile([P, NH], E4, tag="qT8", name="qT8")
    q8 = bigs.tile([HALF, 2, NH], E4, tag="q8", name="q8")
    for s in range(NQCH // 2):
        psq = psb.tile([P, 2, CH], F32, tag="sc", name=f"psq{s}")
        for hh in range(2):
            ch = 2 * s + hh
            nc.tensor.matmul(psq[:, hh, :], lhsT=wq_s[:],
                             rhs=xq_sb[:, ts(ch, CH)])
            nc.vector.tensor_scalar(qT8[:, ts(ch, CH)], psq[:, hh, :],
                                    bq[:, 0:1], None, AOP.add)
            nc.sync.dma_start(q8[:, 0, ts(ch, CH)], qT8[0:HALF, ts(ch, CH)])
            nc.sync.dma_start(q8[:, 1, ts(ch, CH)], qT8[HALF:P, ts(ch, CH)])

    # v in natural [j, c] layout (e4m3); v bias is folded into pb_eff.
    # each psum bank holds one [128, 128] chunk at offset 0 (bank-aligned).
    vnat = bigs.tile([P, NJC, P], E4, tag="vnat", name="vnat")
    for s in range(NJC // 2):          # 16 psum tiles, 2 chunks each
        psv = psb.tile([P, 2, CH], F32, tag="sc", name=f"psv{s}")
        for hh in range(2):
            nc.tensor.matmul(psv[:, hh, 0:P],
                             lhsT=x_sb[:, ts(2 * s + hh, P)], rhs=wv_s[:])
        # batched convert of both chunks (256 elems) on ACT
        nc.scalar.activation(vnat[:, 2 * s:2 * s + 2, :],
                             psv[:, :, 0:P], AFT.Copy)

    # ---------------- attention ----------------
    PT = bigs.tile([P, NJC, CH], E5, tag="PT", name="PT")

    for ib in range(NIB):
        acc = psacc.tile([P, CH], F32, tag="acc", name=f"acc{ib}")
        sm = pssum.tile([P, CH], F32, tag="sp", name=f"sm{ib}")
        qblk = q8[:, :, ts(ib, CH)]
        sched = SCHED[ib]

        def emit_pv(g):
            pair = PT[:, 2 * g:2 * g + 2, :]
            nc.tensor.matmul(
                acc[:], lhsT=vnat[:, 2 * g:2 * g + 2, :], rhs=pair,
                start=(g == 0), stop=(g == NPAIR - 1),
                perf_mode=DR, skip_group_check=True,
            )
            nc.tensor.matmul(
                sm[:], lhsT=ones8[:], rhs=pair,
                start=(g == 0), stop=(g == NPAIR - 1),
                perf_mode=DR, skip_group_check=True,
            )

        for g in range(NPAIR):
            ps = psb.tile([P, 2, CH], F32, tag="sc", name=f"ps{ib}_{g}")
            for hh in range(2):
                jc = 2 * g + hh
                nc.tensor.matmul(ps[:, hh, :],
                                 lhsT=k8[:, :, ts(jc, P)], rhs=qblk,
                                 perf_mode=DR, skip_group_check=True)
            if g > 0:
                emit_pv(g - 1)
            eng = sched[g]
            pair_out = PT[:, 2 * g:2 * g + 2, :]
            if eng == "A":
                nc.scalar.activation(pair_out, ps[:], AFT.Exp, scale=SCALE)
            else:
                nc.vector.tensor_scalar(pair_out.bitcast(I8), ps[:],
                                        A_E, B_E, AOP.mult, AOP.add)
        emit_pv(NPAIR - 1)

        # normalize (v-bias already folded into pb_eff) and project
        recip = work.tile([P, CH], F32, tag="recip", name=f"recip{ib}")
        nc.vector.reciprocal_approx_fast(recip[:], sm[:])
        outn = work.tile([P, CH], F32R, tag="outn", name=f"outn{ib}")
        nc.vector.tensor_mul(outn[:], acc[:], recip[:])

        psp = pssum.tile([P, CH], F32, tag="sp", name=f"psp{ib}")
        nc.tensor.matmul(psp[:], lhsT=wp[:], rhs=outn[:])
        stage = outp.tile([P, CH], F32, tag="stage", name=f"stage{ib}")
        nc.vector.scalar_tensor_tensor(stage[:], psp[:], pbe[:, 0:1],
                                       xq_sb[:, ts(ib, CH)], AOP.add, AOP.add)
        nc.sync.dma_start(out_d.ap()[:, ts(ib, CH)], stage[:])


_NC_CACHE = {}


def _get_nc(reps=1):
    if reps not in _NC_CACHE:
        _NC_CACHE[reps] = _build_program(reps)
    return _NC_CACHE[reps]


def _make_in_maps(x, gn_weight, gn_bias, qkv_weight, qkv_bias, proj_weight,
                  proj_bias):
    x = np.ascontiguousarray(x, dtype=np.float32)
    qkv_weight = np.asarray(qkv_weight, dtype=np.float32)
    qkv_bias = np.asarray(qkv_bias, dtype=np.float32)
    proj_weight = np.asarray(proj_weight, dtype=np.float32)
    proj_bias = np.asarray(proj_bias, dtype=np.float32)
    gn_weight = np.asarray(gn_weight, dtype=np.float32)
    gn_bias = np.asarray(gn_bias, dtype=np.float32)

    b = x.shape[0]
    xf = x.reshape(b, C, N)
    wqT = np.ascontiguousarray(qkv_weight[0:C].T)
    wkT = np.ascontiguousarray(qkv_weight[C:2 * C].T)
    wvT = np.ascontiguousarray(qkv_weight[2 * C:3 * C].T)
    wpT = np.ascontiguousarray(proj_weight.T)
    qkvb2 = np.ascontiguousarray(
        np.stack([qkv_bias[0:C], qkv_bias[C:2 * C]], axis=1))
    bvq = np.ascontiguousarray(qkv_bias[2 * C:3 * C].reshape(C, 1))
    pbv = np.ascontiguousarray(proj_bias.reshape(C, 1))
    gnwv = np.ascontiguousarray(gn_weight.reshape(C, 1))
    gnbv = np.ascontiguousarray(gn_bias.reshape(C, 1))

    in_maps = []
    for core in range(8):
        bi, half = core // 2, core % 2
        in_maps.append({
            "x": np.ascontiguousarray(xf[bi]),
            "xq": np.ascontiguousarray(xf[bi][:, half * NH:(half + 1) * NH]),
            "wqT": wqT, "wkT": wkT, "wvT": wvT, "wpT": wpT,
            "qkvb": qkvb2, "bvq": bvq, "pb": pbv, "gnw": gnwv, "gnb": gnbv,
        })
    return in_maps


def run_on_cores(trace=False, reps=1, **inputs):
    """Build + run on the 8 cores; returns (BassKernelResults, output array)."""
    nc = _get_nc(reps)
    in_maps = _make_in_maps(**inputs)
    res = run_bass_kernel_spmd(nc, in_maps, core_ids=list(range(8)),
                               trace=trace)
    b = np.asarray(inputs["x"]).shape[0]
    h = w = 64
    out = np.empty((b, C, N), dtype=np.float32)
    for core in range(8):
        bi, half = core // 2, core % 2
        out[bi][:, half * NH:(half + 1) * NH] = res.results[core]["out"]
    return res, out.reshape(b, C, h, w)


def kernel(**inputs) -> np.ndarray:
    _, out = run_on_cores(trace=False, **inputs)
    return out


# revision 16
# speedup vs baseline: 2.1092x; 2.1092x over previous
# Trainium2 Bass kernel for nn_Attention_35433480192757
#
# reference computation (b=4, c=128, h=w=64, n=h*w=4096):
#   GroupNorm(8, c) -> 1x1 conv qkv -> full [n, n] attention per batch
#   -> 1x1 conv proj -> residual add
#
# Sharding: 8 cores = 4 batches x 2 query-row halves. Each core computes the
# full k/v for its batch (cheap) and attention for its 2048 query rows.
#
# Speed strategy (hardware-measured on this stack):
#   - scores run as plain-fp8 matmuls (e4m3): the PE streams 2 fp8 moving
#     elements/cycle and FWL makes the per-chunk weight load nearly free
#     (~102 ns per [128x512] matmul measured).
#   - PV and the softmax-sum matmuls use fp8 DoubleRow (2 key-chunks per
#     pass). DoubleRow weight loads are slow (256 cols @ ~1.2GHz), so two
#     i-blocks are fused per pass: each stationary is loaded once and used
#     for 2 matmuls (~95 ns/matmul measured vs 292 unfused).
#   - the softmax-sum matmuls batch at the end of each half behind a single
#     ones-stationary load.
#   - P = exp(scores) is stored e5m2; exp splits across ACT (native Exp)
#     and DVE (one-op Schraudolph bit trick: int8(0.51*raw + 59.78) viewed
#     as e5m2) since GPSIMD/DMA cannot read PSUM on TRN2.
#   - GroupNorm folds into the qkv weights; v bias folds into the proj bias
#     (pb_eff = pb + wp @ bv).

import numpy as np
from contextlib import ExitStack

import concourse.bass as bass
from concourse import bacc
import concourse.tile as tile
import concourse.mybir as mybir
from concourse.bass import ts
from concourse.bass_utils import run_bass_kernel_spmd

P = 128          # partitions == channels
C = 128
N = 4096         # sequence length (h*w) per batch
NH = 2048        # query rows per core
CH = 512         # free-dim chunk (one PSUM bank of fp32)
NCH = N // CH    # 8 column chunks of x
NQCH = NH // CH  # 4 column chunks of xq
NJC = N // P     # 32 key chunks (contraction over j)
NIB = NH // CH   # 4 i-blocks per core
NPAIR = NJC // 2  # 16 key-chunk pairs
NUM_GROUPS = 8
GSIZE = C // NUM_GROUPS
EPS = 1e-5
SCALE = float(C) ** -0.5

F32 = mybir.dt.float32
F32R = mybir.dt.float32r
E4 = mybir.dt.float8e4
E5 = mybir.dt.float8e5
I8 = mybir.dt.int8
AOP = mybir.AluOpType
AFT = mybir.ActivationFunctionType
DR = mybir.MatmulPerfMode.DoubleRow

# e5m2 exp bit trick: bits = round(A_E * raw_score + B_E), bits viewed e5m2.
# A_E folds in the 1/sqrt(c) softmax scale; B_E centres the multiplicative
# sawtooth to match the ACT exact-exp-then-round-to-e5m2 path.
A_E = float(4.0 * np.log2(np.e) * SCALE)
B_E = 60.0 - 0.2171


def _mk_sched(n_d):
    s, acc = [], 0
    for _ in range(NJC):
        acc += n_d
        if acc >= NJC:
            acc -= NJC
            s.append("D")
        else:
            s.append("A")
    return "".join(s)


# exp engine schedule per half: 32 jc slots, 'A' (ACT) or 'D' (DVE)
SCHED = [_mk_sched(15), _mk_sched(13)]


def _build_program(reps=1):
    nc = bacc.Bacc(trn_type="TRN2", num_devices=8)

    x_d = nc.dram_tensor("x", [P, N], F32R, kind="ExternalInput")
    xq_d = nc.dram_tensor("xq", [P, NH], F32R, kind="ExternalInput")
    wqT_d = nc.dram_tensor("wqT", [P, P], F32, kind="ExternalInput")
    wkT_d = nc.dram_tensor("wkT", [P, P], F32, kind="ExternalInput")
    wvT_d = nc.dram_tensor("wvT", [P, P], F32, kind="ExternalInput")
    wpT_d = nc.dram_tensor("wpT", [P, P], F32R, kind="ExternalInput")
    qkvb_d = nc.dram_tensor("qkvb", [P, 2], F32, kind="ExternalInput")
    bvq_d = nc.dram_tensor("bvq", [P, 1], F32, kind="ExternalInput")
    pb_d = nc.dram_tensor("pb", [P, 1], F32, kind="ExternalInput")
    gnw_d = nc.dram_tensor("gnw", [P, 1], F32, kind="ExternalInput")
    gnb_d = nc.dram_tensor("gnb", [P, 1], F32, kind="ExternalInput")
    out_d = nc.dram_tensor("out", [P, NH], F32, kind="ExternalOutput")

    gmat_np = np.zeros((P, P), dtype=np.float32)
    for g in range(NUM_GROUPS):
        gmat_np[g * GSIZE:(g + 1) * GSIZE, g * GSIZE:(g + 1) * GSIZE] = 1.0 / GSIZE
    gmat_d = nc.inline_tensor(gmat_np, "gmat")

    with ExitStack() as ctx:
        tc = ctx.enter_context(tile.TileContext(nc))

        consts = ctx.enter_context(tc.tile_pool(name="consts", bufs=1))
        bigs = ctx.enter_context(tc.tile_pool(name="bigs", bufs=1))
        work = ctx.enter_context(tc.tile_pool(name="work", bufs=2))
        small = ctx.enter_context(tc.tile_pool(name="small", bufs=1))
        outp = ctx.enter_context(tc.tile_pool(name="outp", bufs=2))
        psb = ctx.enter_context(tc.tile_pool(name="psb", bufs=2, space="PSUM"))
        psx = ctx.enter_context(tc.tile_pool(name="psx", bufs=1, space="PSUM"))
        pools = (consts, bigs, work, small, outp, psb, psx)
        drams = (x_d, xq_d, wqT_d, wkT_d, wvT_d, wpT_d, qkvb_d, bvq_d,
                 pb_d, gnw_d, gnb_d, gmat_d, out_d)

        for _rep in range(reps):
            _emit_body(nc, pools, drams)

    nc.compile()
    return nc


def _emit_body(nc, pools, drams):
    (consts, bigs, work, small, outp, psb, psx) = pools
    (x_d, xq_d, wqT_d, wkT_d, wvT_d, wpT_d, qkvb_d, bvq_d,
     pb_d, gnw_d, gnb_d, gmat_d, out_d) = drams

    # ---------------- loads ----------------
    x_sb = bigs.tile([P, N], F32R, tag="x", name="x_sb")
    for s in range(NCH):
        nc.sync.dma_start(x_sb[:, ts(s, CH)], x_d.ap()[:, ts(s, CH)])
    xq_sb = bigs.tile([P, NH], F32R, tag="xq", name="xq_sb")
    for s in range(NQCH):
        nc.sync.dma_start(xq_sb[:, ts(s, CH)], xq_d.ap()[:, ts(s, CH)])

    wq = consts.tile([P, P], F32, tag="wq", name="wq")
    nc.sync.dma_start(wq[:], wqT_d.ap())
    wk = consts.tile([P, P], F32, tag="wk", name="wk")
    nc.sync.dma_start(wk[:], wkT_d.ap())
    wv = consts.tile([P, P], F32, tag="wv", name="wv")
    nc.sync.dma_start(wv[:], wvT_d.ap())
    wp = consts.tile([P, P], F32R, tag="wp", name="wp")
    nc.sync.dma_start(wp[:], wpT_d.ap())
    qkvb = consts.tile([P, 2], F32, tag="qkvb", name="qkvb")
    nc.sync.dma_start(qkvb[:], qkvb_d.ap())
    bvq = consts.tile([P, 1], F32, tag="bvq", name="bvq")
    nc.sync.dma_start(bvq[:], bvq_d.ap())
    pb = consts.tile([P, 1], F32, tag="pb", name="pb")
    nc.sync.dma_start(pb[:], pb_d.ap())
    gnw = consts.tile([P, 1], F32, tag="gnw", name="gnw")
    nc.sync.dma_start(gnw[:], gnw_d.ap())
    gnb = consts.tile([P, 1], F32, tag="gnb", name="gnb")
    nc.sync.dma_start(gnb[:], gnb_d.ap())
    # prefetch Ln and Exp activation tables during the x DMA (1.3us each)
    tdum = small.tile([P, 1], F32, tag="tdum", name="tdum")
    nc.scalar.activation(tdum[:], gnw[:], AFT.Ln)
    nc.scalar.activation(tdum[:], gnw[:], AFT.Exp)
    ones8 = consts.tile([P, 2, P], E5, tag="ones8", name="ones8")
    nc.vector.memset(ones8[:], 1.0)
    # gmat is DMA'd last; the warmup matmul below then observes the DMA-queue
    # semaphore once, so later matmuls need at most one wait (walrus codegen
    # allows only one sync-wait on a self-loading fp32 matmul).
    gmat = consts.tile([P, P], F32, tag="gmat", name="gmat")
    nc.sync.dma_start(gmat[:], gmat_d.ap())

    # PE warmup: absorb the DMA semaphore wait (see note above).
    ps_t = psx.tile([P, CH], F32, tag="smA", name="ps_t")
    nc.tensor.matmul(ps_t[:, 8:10], lhsT=gmat[:], rhs=gmat[:, 0:2])

    # ---------------- GroupNorm stats ----------------
    stats = small.tile([P, NCH, 6], F32, tag="stats", name="stats")
    for s in range(NCH):
        nc.vector.bn_stats(stats[:, s, :], x_sb[:, ts(s, CH)])
    mv = small.tile([P, 2], F32, tag="mv", name="mv")  # per-channel mean, var
    nc.vector.bn_aggr(mv[:], stats[:])

    # t2 = [mean_c, E[x^2]_c]
    t2 = small.tile([P, 2], F32, tag="t2", name="t2")
    nc.vector.tensor_mul(t2[:, 1:2], mv[:, 0:1], mv[:, 0:1])
    nc.vector.tensor_add(t2[:, 1:2], t2[:, 1:2], mv[:, 1:2])
    nc.vector.tensor_copy(t2[:, 0:1], mv[:, 0:1])

    # group-average both stats with the block-diagonal averaging matrix
    nc.tensor.matmul(ps_t[:, 0:2], lhsT=gmat[:], rhs=t2[:])
    gstat = small.tile([P, 2], F32, tag="gstat", name="gstat")
    nc.vector.tensor_copy(gstat[:], ps_t[:, 0:2])

    varv = small.tile([P, 1], F32, tag="varv", name="varv")  # var_g + eps
    nc.vector.tensor_mul(varv[:], gstat[:, 0:1], gstat[:, 0:1])
    nc.vector.tensor_sub(varv[:], gstat[:, 1:2], varv[:])
    nc.vector.tensor_scalar_add(varv[:], varv[:], EPS)

    rstd = small.tile([P, 1], F32, tag="rstd", name="rstd")
    lnv = small.tile([P, 1], F32, tag="lnv", name="lnv")
    nc.scalar.activation(lnv[:], varv[:], AFT.Ln)
    nc.scalar.activation(rstd[:], lnv[:], AFT.Exp, scale=-0.5)
    # two Newton steps to clean up the ACT sqrt approximation
    for it in range(2):
        nt = small.tile([P, 1], F32, tag="nt", name=f"nt{it}")
        nc.vector.tensor_mul(nt[:], rstd[:], rstd[:])
        nc.vector.tensor_mul(nt[:], nt[:], varv[:])
        nc.vector.tensor_scalar(nt[:], nt[:], -0.5, 1.5, AOP.mult, AOP.add)
        nc.vector.tensor_mul(rstd[:], rstd[:], nt[:])

    s_c = small.tile([P, 1], F32, tag="s_c", name="s_c")  # per-channel scale
    nc.vector.tensor_mul(s_c[:], rstd[:], gnw[:])
    t_c = small.tile([P, 1], F32, tag="t_c", name="t_c")  # per-channel shift
    nc.vector.tensor_mul(t_c[:], gstat[:, 0:1], s_c[:])
    nc.vector.tensor_sub(t_c[:], gnb[:], t_c[:])

    # ---------------- fold GN into qkv weights ----------------
    wq_s = consts.tile([P, P], F32R, tag="wq_s", name="wq_s")
    nc.vector.tensor_scalar_mul(wq_s[:], wq[:], s_c[:])
    wk_s = consts.tile([P, P], F32R, tag="wk_s", name="wk_s")
    nc.vector.tensor_scalar_mul(wk_s[:], wk[:], s_c[:])
    wv_s = consts.tile([P, P], F32R, tag="wv_s", name="wv_s")
    nc.vector.tensor_scalar_mul(wv_s[:], wv[:], s_c[:])

    # biases: b_{q,k,v}[o] = (W^T t_c)[o] + qkv_bias[o]
    nc.tensor.matmul(ps_t[:, 2:3], lhsT=wq[:], rhs=t_c[:])
    nc.tensor.matmul(ps_t[:, 3:4], lhsT=wk[:], rhs=t_c[:])
    nc.tensor.matmul(ps_t[:, 6:7], lhsT=wv[:], rhs=t_c[:])
    bq = small.tile([P, 1], F32, tag="bq", name="bq")
    nc.vector.tensor_add(bq[:], ps_t[:, 2:3], qkvb[:, 0:1])
    bk = small.tile([P, 1], F32, tag="bk", name="bk")
    nc.vector.tensor_add(bk[:], ps_t[:, 3:4], qkvb[:, 1:2])
    bv = small.tile([P, 1], F32, tag="bv", name="bv")
    nc.vector.tensor_add(bv[:], ps_t[:, 6:7], bvq[:])
    # fold v bias into proj bias: pb_eff = pb + wp @ bv (plain-f32 matmul)
    wpF = consts.tile([P, P], F32, tag="wpF", name="wpF")
    nc.vector.tensor_copy(wpF[:], wp[:])
    nc.tensor.matmul(ps_t[:, 7:8], lhsT=wpF[:], rhs=bv[:])
    pbe = small.tile([P, 1], F32, tag="pbe", name="pbe")
    nc.vector.tensor_add(pbe[:], ps_t[:, 7:8], pb[:])

    # ---------------- qkv projections ----------------
    kT8 = bigs.tile([P, N], E4, tag="kT8", name="kT8")
    for s in range(NCH // 2):
        psk = psb.tile([P, 2, CH], F32, tag="sc", name=f"psk{s}")
        for hh in range(2):
            ch = 2 * s + hh
            nc.tensor.matmul(psk[:, hh, :], lhsT=wk_s[:],
                             rhs=x_sb[:, ts(ch, CH)])
            # bias-add + e4m3 convert on ACT (Identity allows an AP bias)
            nc.scalar.activation(kT8[:, ts(ch, CH)], psk[:, hh, :],
                                 AFT.Identity, bias=bk[:, 0:1])
    qT8 = bigs.tile([P, NH], E4, tag="qT8", name="qT8")
    for s in range(NQCH // 2):
        psq = psb.tile([P, 2, CH], F32, tag="sc", name=f"psq{s}")
        for hh in range(2):
            ch = 2 * s + hh
            nc.tensor.matmul(psq[:, hh, :], lhsT=wq_s[:],
                             rhs=xq_sb[:, ts(ch, CH)])
            nc.vector.tensor_scalar(qT8[:, ts(ch, CH)], psq[:, hh, :],
                                    bq[:, 0:1], None, AOP.add)

    # v in natural [j, c] layout (e4m3); v bias is folded into pb_eff.
    vnat = bigs.tile([P, NJC, P], E4, tag="vnat", name="vnat")
    for s in range(NJC // 2):          # 16 psum tiles, 2 chunks each
        psv = psb.tile([P, 2, CH], F32, tag="sc", name=f"psv{s}")
        for hh in range(2):
            nc.tensor.matmul(psv[:, hh, 0:P],
                             lhsT=x_sb[:, ts(2 * s + hh, P)], rhs=wv_s[:])
        nc.scalar.activation(vnat[:, 2 * s:2 * s + 2, :],
                             psv[:, :, 0:P], AFT.Copy)

    # ---------------- attention: two halves of 2 fused i-blocks ----------
    for half in range(2):
        ibA, ibB = 2 * half, 2 * half + 1
        accA = psx.tile([P, CH], F32, tag="accA", name=f"accA{half}")
        accB = psx.tile([P, CH], F32, tag="accB", name=f"accB{half}")
        qA = qT8[:, ts(ibA, CH)]
        qB = qT8[:, ts(ibB, CH)]
        PT = bigs.tile([P, NJC, 2, CH], E5, tag="PT", name=f"PT{half}")
        sched = SCHED[half]

        def emit_pv(g):
            vst = vnat[:, 2 * g:2 * g + 2, :]
            nc.tensor.matmul(accA[:], lhsT=vst, rhs=PT[:, 2 * g:2 * g + 2, 0, :],
                             start=(g == 0), stop=(g == NPAIR - 1),
                             perf_mode=DR, skip_group_check=True)
            nc.tensor.matmul(accB[:], lhsT=vst, rhs=PT[:, 2 * g:2 * g + 2, 1, :],
                             start=(g == 0), stop=(g == NPAIR - 1),
                             perf_mode=DR, skip_group_check=True)

        for jc in range(NJC):
            ps = psb.tile([P, 2, CH], F32, tag="sc", name=f"ps{half}_{jc}")
            kst = kT8[:, ts(jc, P)]
            nc.tensor.matmul(ps[:, 0, :], lhsT=kst, rhs=qA,
                             skip_group_check=True)
            nc.tensor.matmul(ps[:, 1, :], lhsT=kst, rhs=qB,
                             skip_group_check=True)
            if jc >= 3 and jc % 2 == 1:
                emit_pv((jc - 3) // 2)
            pair_out = PT[:, jc, :, :]
            if sched[jc] == "A":
                nc.scalar.activation(pair_out, ps[:], AFT.Exp, scale=SCALE)
            else:
                nc.vector.tensor_scalar(pair_out.bitcast(I8), ps[:],
                                        A_E, B_E, AOP.mult, AOP.add)
        emit_pv(NPAIR - 1)

        # softmax-sum matmuls batched behind a single ones-stationary load
        smA = psx.tile([P, CH], F32, tag="smA", name=f"smA{half}")
        smB = psx.tile([P, CH], F32, tag="smB", name=f"smB{half}")
        for g in range(NPAIR):
            nc.tensor.matmul(smA[:], lhsT=ones8[:],
                             rhs=PT[:, 2 * g:2 * g + 2, 0, :],
                             start=(g == 0), stop=(g == NPAIR - 1),
                             perf_mode=DR, skip_group_check=True)
        for g in range(NPAIR):
            nc.tensor.matmul(smB[:], lhsT=ones8[:],
                             rhs=PT[:, 2 * g:2 * g + 2, 1, :],
                             start=(g == 0), stop=(g == NPAIR - 1),
                             perf_mode=DR, skip_group_check=True)

        for ib, acc, sm in ((ibA, accA, smA), (ibB, accB, smB)):
            recip = work.tile([P, CH], F32, tag="recip", name=f"recip{ib}")
            nc.vector.reciprocal_approx_fast(recip[:], sm[:])
            outn = work.tile([P, CH], F32R, tag="outn", name=f"outn{ib}")
            nc.vector.tensor_mul(outn[:], acc[:], recip[:])
            tag = "smA" if ib == ibA else "smB"
            psp = psx.tile([P, CH], F32, tag=tag, name=f"psp{ib}")
            nc.tensor.matmul(psp[:], lhsT=wp[:], rhs=outn[:])
            stage = outp.tile([P, CH], F32, tag="stage", name=f"stage{ib}")
            nc.vector.scalar_tensor_tensor(stage[:], psp[:], pbe[:, 0:1],
                                           xq_sb[:, ts(ib, CH)],
                                           AOP.add, AOP.add)
            nc.sync.dma_start(out_d.ap()[:, ts(ib, CH)], stage[:])


_NC_CACHE = {}


def _get_nc(reps=1):
    if reps not in _NC_CACHE:
        _NC_CACHE[reps] = _build_program(reps)
    return _NC_CACHE[reps]


def _make_in_maps(x, gn_weight, gn_bias, qkv_weight, qkv_bias, proj_weight,
                  proj_bias):
    x = np.ascontiguousarray(x, dtype=np.float32)
    qkv_weight = np.asarray(qkv_weight, dtype=np.float32)
    qkv_bias = np.asarray(qkv_bias, dtype=np.float32)
    proj_weight = np.asarray(proj_weight, dtype=np.float32)
    proj_bias = np.asarray(proj_bias, dtype=np.float32)
    gn_weight = np.asarray(gn_weight, dtype=np.float32)
    gn_bias = np.asarray(gn_bias, dtype=np.float32)

    b = x.shape[0]
    xf = x.reshape(b, C, N)
    wqT = np.ascontiguousarray(qkv_weight[0:C].T)
    wkT = np.ascontiguousarray(qkv_weight[C:2 * C].T)
    wvT = np.ascontiguousarray(qkv_weight[2 * C:3 * C].T)
    wpT = np.ascontiguousarray(proj_weight.T)
    qkvb2 = np.ascontiguousarray(
        np.stack([qkv_bias[0:C], qkv_bias[C:2 * C]], axis=1))
    bvq = np.ascontiguousarray(qkv_bias[2 * C:3 * C].reshape(C, 1))
    pbv = np.ascontiguousarray(proj_bias.reshape(C, 1))
    gnwv = np.ascontiguousarray(gn_weight.reshape(C, 1))
    gnbv = np.ascontiguousarray(gn_bias.reshape(C, 1))

    in_maps = []
    for core in range(8):
        bi, half = core // 2, core % 2
        in_maps.append({
            "x": np.ascontiguousarray(xf[bi]),
            "xq": np.ascontiguousarray(xf[bi][:, half * NH:(half + 1) * NH]),
            "wqT": wqT, "wkT": wkT, "wvT": wvT, "wpT": wpT,
            "qkvb": qkvb2, "bvq": bvq, "pb": pbv, "gnw": gnwv, "gnb": gnbv,
        })
    return in_maps


def run_on_cores(trace=False, reps=1, **inputs):
    """Build + run on the 8 cores; returns (BassKernelResults, output array)."""
    nc = _get_nc(reps)
    in_maps = _make_in_maps(**inputs)
    res = run_bass_kernel_spmd(nc, in_maps, core_ids=list(range(8)),
                               trace=trace)
    b = np.asarray(inputs["x"]).shape[0]
    h = w = 64
    out = np.empty((b, C, N), dtype=np.float32)
    for core in range(8):
        bi, half = core // 2, core % 2
        out[bi][:, half * NH:(half + 1) * NH] = res.results[core]["out"]
    return res, out.reshape(b, C, h, w)


def kernel(**inputs) -> np.ndarray:
    _, out = run_on_cores(trace=False, **inputs)
    return out
